# revision 5
# baseline (speedup 1.0000x reference)
"""FP4-LUT dequant + GEMM kernel for Trainium2 (8 NeuronCores).

Computes y = x @ W^T where W[n,k] = lut[fp4_idx[n,k]] is packed two
nibbles per byte (high nibble = even k, low = odd k), x fp16 [M,K],
y fp16 [M,N], fp32 accumulation.

Sharding: column-parallel. Core c owns y[:, 512c:512c+512]; x is
replicated, packed_weight rows [512c:512c+512] go to core c.

Device-side pipeline per core:
  1. Dequant: packed bytes (uint16) -> fp16 W values via fused integer
     bit-assembly on the vector engine (exact, no LUT gather needed),
     written nk-interleaved to a DRAM scratch W[n, k].
  2. W transpose: dma_start_transpose DRAM->SBUF gives WT[k, n] tiles.
  3. GEMM: psum[m,n] += xT[k,m].T @ WT[k,n]; xT tiles come from
     dma_start_transpose of x. m-groups of 8 PSUM banks x 4 k-phases so
     the PE starts consuming k-tiles while later ones still dequantize.
"""

import numpy as np

import concourse.bacc as bacc
import concourse.mybir as mybir
from concourse import tile
from concourse.alu_op_type import AluOpType as alu
from concourse.bass_utils import run_bass_kernel_spmd

M = 4096
K = 4096
N = 4096
N_CORES = 8
N_SHARD = N // N_CORES  # 512

# The exact LUT this kernel's bit-assembly decode implements.
FP4_E2M1_VALUES = [0.0, 0.0625, 8.0, 1.0, 2.0, 3.0, 4.0, 6.0,
                   -0.0, -0.0625, -8.0, -1.0, -2.0, -3.0, -4.0, -6.0]

U16 = mybir.dt.uint16
F16 = mybir.dt.float16
F32 = mybir.dt.float32


def _decode_plane(nc, pool, q, sgn, out_ap, fd):
    """Assemble fp16 bit patterns for lut[p] from q = p&7, sgn = (p>>3)<<15.

    t(q) = (q>=3)*(q + (q>=4) + 27) + 22*(q==1) + 36*(q==2)
    gives t = [0, 22, 36, 30, 32, 33, 34, 35]; the fp16 encoding is
    t*512 + sgn (t<512 and sgn is bit 15, so add == or), which decodes
    lut = [0, 0.0625, 8, 1, 2, 3, 4, 6] with sign from the high bit.
    All intermediates are small non-negative ints (no wraparound).
    """
    a = pool.tile([128, fd], U16, tag="dq_a")
    a27 = pool.tile([128, fd], U16, tag="dq_a27")
    th = pool.tile([128, fd], U16, tag="dq_th")
    e1 = pool.tile([128, fd], U16, tag="dq_e1")
    e2 = pool.tile([128, fd], U16, tag="dq_e2")
    t0 = pool.tile([128, fd], U16, tag="dq_t0")
    t1 = pool.tile([128, fd], U16, tag="dq_t1")
    v = nc.vector
    v.scalar_tensor_tensor(a[:], q[:], 4, q[:], op0=alu.is_ge, op1=alu.add)
    v.tensor_scalar(a27[:], a[:], 27, 0, op0=alu.add, op1=alu.add)
    v.scalar_tensor_tensor(th[:], q[:], 3, a27[:], op0=alu.is_ge, op1=alu.mult)
    v.tensor_scalar(e1[:], q[:], 1, 22, op0=alu.is_equal, op1=alu.mult)
    v.tensor_scalar(e2[:], q[:], 2, 36, op0=alu.is_equal, op1=alu.mult)
    v.tensor_tensor(t0[:], th[:], e1[:], op=alu.add)
    v.tensor_tensor(t1[:], t0[:], e2[:], op=alu.add)
    v.scalar_tensor_tensor(out_ap, t1[:], 512, sgn[:], op0=alu.mult, op1=alu.add)


def build_nc(m=M, k=K, n_shard=N_SHARD):
    """Build the per-core bass module (SPMD: same program on all cores)."""
    kh = k // 2
    n_kt = k // 128           # k-tiles of 128
    n_mt = m // 128           # m-tiles of 128
    n_nt = n_shard // 128     # packed-weight row tiles
    # dequant chunking: j-chunks of 512 bytes -> k-phases of 8 k-tiles
    jc_w = 512
    n_jc = kh // jc_w
    kt_per_jc = n_kt // n_jc
    # GEMM grouping: groups of 8 m-tiles (8 psum banks), phases = jc
    mt_per_g = min(8, n_mt)
    n_g = n_mt // mt_per_g

    nc = bacc.Bacc("TRN2", target_bir_lowering=False, debug=False)
    x = nc.dram_tensor("x", [m, k], F16, kind="ExternalInput")
    pw = nc.dram_tensor("pw", [n_shard, kh], U16, kind="ExternalInput")
    y = nc.dram_tensor("y", [m, n_shard], F16, kind="ExternalOutput")

    with tile.TileContext(nc) as tc:
        with (
            tc.tile_pool(name="dram", bufs=1, space="DRAM") as dram_pool,
            tc.tile_pool(name="wt", bufs=1) as wt_pool,
            tc.tile_pool(name="pwp", bufs=3) as pw_pool,
            tc.tile_pool(name="dq", bufs=3) as dq_pool,
            tc.tile_pool(name="wnat", bufs=3) as wnat_pool,
            tc.tile_pool(name="xt", bufs=20) as xt_pool,
            tc.tile_pool(name="psum", bufs=8, space="PSUM") as psum_pool,
            tc.tile_pool(name="out", bufs=4) as out_pool,
        ):
            # ---- Dequant + W transpose, pipelined by j-chunk ----
            wdram = [
                dram_pool.tile([n_shard, 2 * jc_w], U16, tag=f"wdram{jc}", name=f"wdram{jc}")
                for jc in range(n_jc)
            ]
            wt_tiles = []
            for kt in range(n_kt):
                wt_tiles.append(wt_pool.tile([128, n_shard], U16, tag=f"wt{kt}", name=f"wt{kt}"))

            for jc in range(n_jc):
                for nt in range(n_nt):
                    b = pw_pool.tile([128, jc_w], U16, tag="pwb")
                    nc.sync.dma_start(
                        out=b[:],
                        in_=pw[nt * 128:(nt + 1) * 128, jc * jc_w:(jc + 1) * jc_w],
                    )
                    wnat = wnat_pool.tile([128, 2 * jc_w], U16, tag="wnat")
                    v = nc.vector
                    # hi nibble -> even k
                    q = dq_pool.tile([128, jc_w], U16, tag="dq_q")
                    sgn = dq_pool.tile([128, jc_w], U16, tag="dq_s")
                    v.tensor_scalar(
                        q[:], b[:], 4, 7,
                        op0=alu.logical_shift_right, op1=alu.bitwise_and,
                    )
                    v.tensor_scalar(
                        sgn[:], b[:], 7, 15,
                        op0=alu.logical_shift_right, op1=alu.logical_shift_left,
                    )
                    _decode_plane(nc, dq_pool, q, sgn, wnat[:, 0::2], jc_w)
                    # lo nibble -> odd k
                    q2 = dq_pool.tile([128, jc_w], U16, tag="dq_q")
                    sgn2 = dq_pool.tile([128, jc_w], U16, tag="dq_s")
                    v.tensor_scalar(
                        q2[:], b[:], 7, 0, op0=alu.bitwise_and, op1=alu.bitwise_or
                    )
                    v.tensor_scalar(
                        sgn2[:], b[:], 8, 12, op0=alu.bitwise_and,
                        op1=alu.logical_shift_left,
                    )
                    _decode_plane(nc, dq_pool, q2, sgn2, wnat[:, 1::2], jc_w)
                    nc.sync.dma_start(
                        out=wdram[jc][nt * 128:(nt + 1) * 128, :], in_=wnat[:]
                    )
                for kl in range(kt_per_jc):
                    kt = jc * kt_per_jc + kl
                    nc.sync.dma_start_transpose(
                        wt_tiles[kt][:], wdram[jc][:, kl * 128:(kl + 1) * 128]
                    )

            # ---- GEMM ----
            for g in range(n_g):
                psums = [
                    psum_pool.tile([128, n_shard], F32, tag="ps", name="ps")
                    for _ in range(mt_per_g)
                ]
                for p in range(n_jc):
                    xts = []
                    for kl in range(kt_per_jc):
                        kt = p * kt_per_jc + kl
                        halves = []
                        for h in range(mt_per_g // 4):
                            xt = xt_pool.tile([128, 512], F16, tag="xt")
                            m0 = g * (mt_per_g * 128) + h * 512
                            nc.sync.dma_start_transpose(
                                xt[:], x[m0:m0 + 512, kt * 128:(kt + 1) * 128]
                            )
                            halves.append(xt)
                        xts.append(halves)
                    for ml in range(mt_per_g):
                        for kl in range(kt_per_jc):
                            kt = p * kt_per_jc + kl
                            lhsT = xts[kl][ml // 4][:, (ml % 4) * 128:(ml % 4 + 1) * 128]
                            nc.tensor.matmul(
                                psums[ml][:],
                                lhsT,
                                wt_tiles[kt][:].bitcast(F16),
                                start=(p == 0 and kl == 0),
                                stop=(p == n_jc - 1 and kl == kt_per_jc - 1),
                            )
                for ml in range(mt_per_g):
                    ot = out_pool.tile([128, n_shard], F16, tag="ot")
                    nc.vector.tensor_copy(out=ot[:], in_=psums[ml][:])
                    mt = g * mt_per_g + ml
                    nc.sync.dma_start(
                        out=y[mt * 128:(mt + 1) * 128, :], in_=ot[:]
                    )
    nc.compile()
    return nc


_NC_CACHE = {}


def _run(x, packed_weight, **spmd_kwargs):
    key = "full"
    if key not in _NC_CACHE:
        _NC_CACHE[key] = build_nc()
    nc = _NC_CACHE[key]

    x = np.ascontiguousarray(np.asarray(x, dtype=np.float16))
    pw_u16 = np.asarray(packed_weight, dtype=np.int32).astype(np.uint16)
    in_maps = [
        {
            "x": x,
            "pw": np.ascontiguousarray(
                pw_u16[c * N_SHARD:(c + 1) * N_SHARD, :]
            ),
        }
        for c in range(N_CORES)
    ]
    res = run_bass_kernel_spmd(
        nc, in_maps, core_ids=list(range(N_CORES)), **spmd_kwargs
    )
    y = np.concatenate([res.results[c]["y"] for c in range(N_CORES)], axis=1)
    return y, res


def kernel(x, packed_weight, lut):
    assert np.allclose(np.asarray(lut, np.float32),
                       np.array(FP4_E2M1_VALUES, np.float32)), \
        "kernel's hardcoded decode only supports the standard table"
    y, _ = _run(x, packed_weight)
    return y


# revision 14
# speedup vs baseline: 1.6368x; 1.6368x over previous
"""FP4-LUT dequant + GEMM kernel for Trainium2 (8 NeuronCores).

Computes y = x @ W^T where W[n,k] = lut[fp4_idx[n,k]] is packed two
nibbles per byte (high nibble = even k, low = odd k), x fp16 [M,K],
y fp16 [M,N], fp32 accumulation.

Sharding: column-parallel. Core c owns y[:, 512c:512c+512]; x is
replicated, packed_weight rows [512c:512c+512] go to core c.

Device-side pipeline per core:
  1. Dequant: packed bytes (uint16) -> fp16 W values via fused integer
     bit-assembly on the vector/gpsimd engines (exact, no LUT gather),
     written nk-interleaved to a DRAM scratch W[n, k].
  2. W transpose: dma_start_transpose DRAM->SBUF gives WT[k, n] slabs.
  3. GEMM: psum[m,n] += xT[k,m].T @ WT[k,n]; xT slabs come from batched
     dma_start_transpose of x, alternating the two HWDGE queues
     (sync/scalar). m-groups of 8 PSUM banks x 4 k-phases so the PE
     starts consuming k-tiles while later ones still dequantize.
"""

import numpy as np

import concourse.bacc as bacc
import concourse.mybir as mybir
from concourse import tile
from concourse.alu_op_type import AluOpType as alu
from concourse.bass_utils import run_bass_kernel_spmd

M = 4096
K = 4096
N = 4096
N_CORES = 8
N_SHARD = N // N_CORES  # 512

# The exact LUT this kernel's bit-assembly decode implements.
FP4_E2M1_VALUES = [0.0, 0.0625, 8.0, 1.0, 2.0, 3.0, 4.0, 6.0,
                   -0.0, -0.0625, -8.0, -1.0, -2.0, -3.0, -4.0, -6.0]

U16 = mybir.dt.uint16
F16 = mybir.dt.float16
F32 = mybir.dt.float32


def _decode_chunk(eng, pool, b, wnat, fd):
    """Decode a [128, fd] byte tile into wnat [128, 2*fd] fp16-bit values,
    laid out [hi-plane | lo-plane] (the DRAM write interleaves to k order).

    For nibble p (q = p&7, s = p>>3), the fp16 bit pattern of lut[p] is
        bits = 512*t(q) + (s<<15),
        t(q) = (q>=3)*(q + 27 + (q>=4)) + 22*(q==1) + 36*(q==2)
    i.e. t = [0, 22, 36, 30, 32, 33, 34, 35], decoding
    lut = [0, 0.0625, 8, 1, 2, 3, 4, 6] with sign from the high bit.
    All ops are tensor_scalar/tensor_tensor (2x DVE mode); constants are
    pre-scaled by 512 so no final shift is needed; everything is small
    non-negative ints (no wraparound).
    """
    f2 = 2 * fd
    q = pool.tile([128, f2], U16, tag="dq_q", name="dq_q")
    sg = pool.tile([128, f2], U16, tag="dq_sg", name="dq_sg")
    m4 = pool.tile([128, f2], U16, tag="dq_m4", name="dq_m4")
    r = pool.tile([128, f2], U16, tag="dq_r", name="dq_r")
    r2 = pool.tile([128, f2], U16, tag="dq_r2", name="dq_r2")
    m3 = pool.tile([128, f2], U16, tag="dq_m3", name="dq_m3")
    th = pool.tile([128, f2], U16, tag="dq_th", name="dq_th")
    e1 = pool.tile([128, f2], U16, tag="dq_e1", name="dq_e1")
    e2 = pool.tile([128, f2], U16, tag="dq_e2", name="dq_e2")
    t0 = pool.tile([128, f2], U16, tag="dq_t0", name="dq_t0")
    t1 = pool.tile([128, f2], U16, tag="dq_t1", name="dq_t1")
    # nibble index p into q halves, sign bits into sg halves
    eng.tensor_scalar(q[:, :fd], b[:], 4, 7,
                      op0=alu.logical_shift_right, op1=alu.bitwise_and)
    eng.tensor_scalar(q[:, fd:], b[:], 7, 0,
                      op0=alu.bitwise_and, op1=alu.bitwise_or)
    eng.tensor_scalar(sg[:, :fd], b[:], 7, 15,
                      op0=alu.logical_shift_right, op1=alu.logical_shift_left)
    eng.tensor_scalar(sg[:, fd:], b[:], 8, 12,
                      op0=alu.bitwise_and, op1=alu.logical_shift_left)
    # t*512 assembly, all at full fd=2*fd width
    eng.tensor_scalar(m4[:], q[:], 4, 512, op0=alu.is_ge, op1=alu.mult)
    eng.tensor_scalar(r[:], q[:], 512, 27 * 512, op0=alu.mult, op1=alu.add)
    eng.tensor_tensor(r2[:], r[:], m4[:], op=alu.add)
    eng.tensor_scalar(m3[:], q[:], 3, 1, op0=alu.is_ge, op1=alu.mult)
    eng.tensor_tensor(th[:], m3[:], r2[:], op=alu.mult)
    eng.tensor_scalar(e1[:], q[:], 1, 22 * 512, op0=alu.is_equal, op1=alu.mult)
    eng.tensor_scalar(e2[:], q[:], 2, 36 * 512, op0=alu.is_equal, op1=alu.mult)
    eng.tensor_tensor(t0[:], th[:], e1[:], op=alu.add)
    eng.tensor_tensor(t1[:], t0[:], e2[:], op=alu.add)
    # add sign bit and interleave planes to natural k order (strided out)
    eng.tensor_tensor(wnat[:, 0::2], t1[:, :fd], sg[:, :fd], op=alu.add)
    eng.tensor_tensor(wnat[:, 1::2], t1[:, fd:], sg[:, fd:], op=alu.add)


def build_nc(m=M, k=K, n_shard=N_SHARD):
    """Build the per-core bass module (SPMD: same program on all cores)."""
    kh = k // 2
    n_kt = k // 128           # k-tiles of 128
    n_mt = m // 128           # m-tiles of 128
    n_nt = n_shard // 128     # packed-weight row tiles
    # dequant chunking: j-chunks of 512 bytes -> k-phases of 8 k-tiles
    jc_w = 512
    n_jc = kh // jc_w
    kt_per_jc = n_kt // n_jc
    # GEMM grouping: groups of 8 m-tiles (8 psum banks), phases = jc
    mt_per_g = min(8, n_mt)
    n_g = n_mt // mt_per_g

    nc = bacc.Bacc("TRN2", target_bir_lowering=False, debug=False)
    x = nc.dram_tensor("x", [m, k], F16, kind="ExternalInput")
    pw = nc.dram_tensor("pw", [n_shard, kh], U16, kind="ExternalInput")
    y = nc.dram_tensor("y", [m, n_shard], F16, kind="ExternalOutput")

    hwdge = [nc.sync, nc.sync]

    with tile.TileContext(nc) as tc:
        with (
            tc.tile_pool(name="dram", bufs=1, space="DRAM") as dram_pool,
            tc.tile_pool(name="wt", bufs=1) as wt_pool,
            tc.tile_pool(name="pwp", bufs=3) as pw_pool,
            tc.tile_pool(name="dqv", bufs=3) as dqv_pool,
            tc.tile_pool(name="dqg", bufs=3) as dqg_pool,
            tc.tile_pool(name="wnat", bufs=4) as wnat_pool,
            tc.tile_pool(name="xt", bufs=6) as xt_pool,
            tc.tile_pool(name="psum", bufs=8, space="PSUM") as psum_pool,
            tc.tile_pool(name="out", bufs=4) as out_pool,
        ):
            # ---- Dequant + W transpose, pipelined by j-chunk ----
            wdram = [
                dram_pool.tile([n_shard, 2 * jc_w], U16,
                               tag=f"wdram{jc}", name=f"wdram{jc}")
                for jc in range(n_jc)
            ]
            wt_slabs = [
                wt_pool.tile([128, kt_per_jc, n_shard], U16,
                             tag=f"wts{jc}", name=f"wts{jc}")
                for jc in range(n_jc)
            ]

            for jc in range(n_jc):
                for nt in range(n_nt):
                    b = pw_pool.tile([128, jc_w], U16, tag="pwb", name="pwb")
                    nc.gpsimd.dma_start(
                        out=b[:],
                        in_=pw[nt * 128:(nt + 1) * 128, jc * jc_w:(jc + 1) * jc_w],
                    )
                    wnat = wnat_pool.tile([128, 2 * jc_w], U16,
                                          tag="wnat", name="wnat")
                    _decode_chunk(nc.vector, dqv_pool, b, wnat, jc_w)
                    # interleave [hi | lo] planes to natural k order
                    # (even k from hi, odd k from lo) during the DRAM write
                    nc.gpsimd.dma_start(
                        out=wdram[jc][nt * 128:(nt + 1) * 128, :], in_=wnat[:]
                    )
                hwdge[jc % 2].dma_start_transpose(wt_slabs[jc][:], wdram[jc][:])

            # ---- GEMM ----
            for g in range(n_g):
                psums = [
                    psum_pool.tile([128, n_shard], F32, tag="ps", name="ps")
                    for _ in range(mt_per_g)
                ]
                for p in range(n_jc):
                    xts = []
                    for h in range(mt_per_g // 4):
                        xt = xt_pool.tile([128, kt_per_jc, 512], F16,
                                          tag="xt", name="xt")
                        m0 = g * (mt_per_g * 128) + h * 512
                        k0 = p * (kt_per_jc * 128)
                        hwdge[(g * 2 + h) % 2].dma_start_transpose(
                            xt[:], x[m0:m0 + 512, k0:k0 + kt_per_jc * 128]
                        )
                        xts.append(xt)
                    for ml in range(mt_per_g):
                        for kl in range(kt_per_jc):
                            lhsT = xts[ml // 4][:, kl,
                                                (ml % 4) * 128:(ml % 4 + 1) * 128]
                            nc.tensor.matmul(
                                psums[ml][:],
                                lhsT,
                                wt_slabs[p][:, kl, :].bitcast(F16),
                                start=(p == 0 and kl == 0),
                                stop=(p == n_jc - 1 and kl == kt_per_jc - 1),
                            )
                for ml in range(mt_per_g):
                    ot = out_pool.tile([128, n_shard], F16, tag="ot", name="ot")
                    nc.vector.tensor_copy(out=ot[:], in_=psums[ml][:])
                    mt = g * mt_per_g + ml
                    nc.gpsimd.dma_start(
                        out=y[mt * 128:(mt + 1) * 128, :], in_=ot[:]
                    )
    nc.compile()
    return nc


_NC_CACHE = {}


def _run(x, packed_weight, **spmd_kwargs):
    key = "full"
    if key not in _NC_CACHE:
        _NC_CACHE[key] = build_nc()
    nc = _NC_CACHE[key]

    x = np.ascontiguousarray(np.asarray(x, dtype=np.float16))
    pw_u16 = np.asarray(packed_weight, dtype=np.int32).astype(np.uint16)
    in_maps = [
        {
            "x": x,
            "pw": np.ascontiguousarray(
                pw_u16[c * N_SHARD:(c + 1) * N_SHARD, :]
            ),
        }
        for c in range(N_CORES)
    ]
    res = run_bass_kernel_spmd(
        nc, in_maps, core_ids=list(range(N_CORES)), **spmd_kwargs
    )
    y = np.concatenate([res.results[c]["y"] for c in range(N_CORES)], axis=1)
    return y, res


def kernel(x, packed_weight, lut):
    assert np.allclose(np.asarray(lut, np.float32),
                       np.array(FP4_E2M1_VALUES, np.float32)), \
        "kernel's hardcoded decode only supports the standard table"
    y, _ = _run(x, packed_weight)
    return y


# revision 15
# speedup vs baseline: 1.7368x; 1.0611x over previous
"""FP4-LUT dequant + GEMM kernel for Trainium2 (8 NeuronCores).

Computes y = x @ W^T where W[n,k] = lut[fp4_idx[n,k]] is packed two
nibbles per byte (high nibble = even k, low = odd k), x fp16 [M,K],
y fp16 [M,N], fp32 accumulation.

Sharding: column-parallel. Core c owns y[:, 512c:512c+512]; x is
replicated, packed_weight rows [512c:512c+512] go to core c.

Device-side pipeline per core:
  1. Dequant: packed bytes (uint16) -> fp16 W values via fused integer
     bit-assembly on the vector/gpsimd engines (exact, no LUT gather),
     written nk-interleaved to a DRAM scratch W[n, k].
  2. W transpose: dma_start_transpose DRAM->SBUF gives WT[k, n] slabs.
  3. GEMM: psum[m,n] += xT[k,m].T @ WT[k,n]; xT slabs come from batched
     dma_start_transpose of x, alternating the two HWDGE queues
     (sync/scalar). m-groups of 8 PSUM banks x 4 k-phases so the PE
     starts consuming k-tiles while later ones still dequantize.
"""

import numpy as np

import concourse.bacc as bacc
import concourse.mybir as mybir
from concourse import tile
from concourse.alu_op_type import AluOpType as alu
from concourse.bass_utils import run_bass_kernel_spmd

M = 4096
K = 4096
N = 4096
N_CORES = 8
N_SHARD = N // N_CORES  # 512

# The exact LUT this kernel's bit-assembly decode implements.
FP4_E2M1_VALUES = [0.0, 0.0625, 8.0, 1.0, 2.0, 3.0, 4.0, 6.0,
                   -0.0, -0.0625, -8.0, -1.0, -2.0, -3.0, -4.0, -6.0]

U16 = mybir.dt.uint16
F16 = mybir.dt.float16
F32 = mybir.dt.float32


def _decode_chunk(eng, pool, b, wnat, fd):
    """Decode a [128, fd] byte tile into wnat [128, 2*fd] fp16-bit values,
    laid out [hi-plane | lo-plane] (the DRAM write interleaves to k order).

    For nibble p (q = p&7, s = p>>3), the fp16 bit pattern of lut[p] is
        bits = 512*t(q) + (s<<15),
        t(q) = (q>=3)*(q + 27 + (q>=4)) + 22*(q==1) + 36*(q==2)
    i.e. t = [0, 22, 36, 30, 32, 33, 34, 35], decoding
    lut = [0, 0.0625, 8, 1, 2, 3, 4, 6] with sign from the high bit.
    All ops are tensor_scalar/tensor_tensor (2x DVE mode); constants are
    pre-scaled by 512 so no final shift is needed; everything is small
    non-negative ints (no wraparound).
    """
    f2 = 2 * fd
    q = pool.tile([128, f2], U16, tag="dq_q", name="dq_q")
    sg = pool.tile([128, f2], U16, tag="dq_sg", name="dq_sg")
    m4 = pool.tile([128, f2], U16, tag="dq_m4", name="dq_m4")
    r = pool.tile([128, f2], U16, tag="dq_r", name="dq_r")
    r2 = pool.tile([128, f2], U16, tag="dq_r2", name="dq_r2")
    m3 = pool.tile([128, f2], U16, tag="dq_m3", name="dq_m3")
    th = pool.tile([128, f2], U16, tag="dq_th", name="dq_th")
    e1 = pool.tile([128, f2], U16, tag="dq_e1", name="dq_e1")
    e2 = pool.tile([128, f2], U16, tag="dq_e2", name="dq_e2")
    t0 = pool.tile([128, f2], U16, tag="dq_t0", name="dq_t0")
    t1 = pool.tile([128, f2], U16, tag="dq_t1", name="dq_t1")
    # nibble index p into q halves, sign bits into sg halves
    eng.tensor_scalar(q[:, :fd], b[:], 4, 7,
                      op0=alu.logical_shift_right, op1=alu.bitwise_and)
    eng.tensor_scalar(q[:, fd:], b[:], 7, 0,
                      op0=alu.bitwise_and, op1=alu.bitwise_or)
    eng.tensor_scalar(sg[:, :fd], b[:], 7, 15,
                      op0=alu.logical_shift_right, op1=alu.logical_shift_left)
    eng.tensor_scalar(sg[:, fd:], b[:], 8, 12,
                      op0=alu.bitwise_and, op1=alu.logical_shift_left)
    # t*512 assembly, all at full fd=2*fd width
    eng.tensor_scalar(m4[:], q[:], 4, 512, op0=alu.is_ge, op1=alu.mult)
    eng.tensor_scalar(r[:], q[:], 512, 27 * 512, op0=alu.mult, op1=alu.add)
    eng.tensor_tensor(r2[:], r[:], m4[:], op=alu.add)
    eng.tensor_scalar(m3[:], q[:], 3, 1, op0=alu.is_ge, op1=alu.mult)
    eng.tensor_tensor(th[:], m3[:], r2[:], op=alu.mult)
    eng.tensor_scalar(e1[:], q[:], 1, 22 * 512, op0=alu.is_equal, op1=alu.mult)
    eng.tensor_scalar(e2[:], q[:], 2, 36 * 512, op0=alu.is_equal, op1=alu.mult)
    eng.tensor_tensor(t0[:], th[:], e1[:], op=alu.add)
    eng.tensor_tensor(t1[:], t0[:], e2[:], op=alu.add)
    # add sign bit and interleave planes to natural k order (strided out)
    eng.tensor_tensor(wnat[:, 0::2], t1[:, :fd], sg[:, :fd], op=alu.add)
    eng.tensor_tensor(wnat[:, 1::2], t1[:, fd:], sg[:, fd:], op=alu.add)


def build_nc(m=M, k=K, n_shard=N_SHARD):
    """Build the per-core bass module (SPMD: same program on all cores)."""
    kh = k // 2
    n_kt = k // 128           # k-tiles of 128
    n_mt = m // 128           # m-tiles of 128
    n_nt = n_shard // 128     # packed-weight row tiles
    # dequant chunking: j-chunks of 512 bytes -> k-phases of 8 k-tiles
    jc_w = 512
    n_jc = kh // jc_w
    kt_per_jc = n_kt // n_jc
    # GEMM grouping: groups of 8 m-tiles (8 psum banks), phases = jc
    mt_per_g = min(8, n_mt)
    n_g = n_mt // mt_per_g

    nc = bacc.Bacc("TRN2", target_bir_lowering=False, debug=False)
    x = nc.dram_tensor("x", [m, k], F16, kind="ExternalInput")
    pw = nc.dram_tensor("pw", [n_shard, kh], U16, kind="ExternalInput")
    y = nc.dram_tensor("y", [m, n_shard], F16, kind="ExternalOutput")

    hwdge = [nc.sync, nc.sync]

    # semi-passes: split k-phases in two so each m-group only needs half
    # of W per pass; partial sums stage in SBUF fp32. This lets the PE
    # start long before dequant finishes.
    half = n_jc // 2
    with tile.TileContext(nc) as tc:
        with (
            tc.tile_pool(name="dram", bufs=1, space="DRAM") as dram_pool,
            tc.tile_pool(name="wt", bufs=1) as wt_pool,
            tc.tile_pool(name="pwp", bufs=3) as pw_pool,
            tc.tile_pool(name="dqv", bufs=2) as dqv_pool,
            tc.tile_pool(name="wnat", bufs=4) as wnat_pool,
            tc.tile_pool(name="xt", bufs=3) as xt_pool,
            tc.tile_pool(name="stg", bufs=1) as stg_pool,
            tc.tile_pool(name="psum", bufs=8, space="PSUM") as psum_pool,
            tc.tile_pool(name="out", bufs=4) as out_pool,
        ):
            # ---- Dequant (vector + gpsimd DMA only; HWDGE queue left
            # free for the transposes) ----
            wdram = [
                dram_pool.tile([n_shard, 2 * jc_w], U16,
                               tag=f"wdram{jc}", name=f"wdram{jc}")
                for jc in range(n_jc)
            ]
            wt_slabs = [
                wt_pool.tile([128, kt_per_jc, n_shard], U16,
                             tag=f"wts{jc}", name=f"wts{jc}")
                for jc in range(n_jc)
            ]
            stg = [
                [
                    stg_pool.tile([128, n_shard], F32,
                                  tag=f"stg{g}_{ml}", name=f"stg{g}_{ml}")
                    for ml in range(mt_per_g)
                ]
                for g in range(n_g)
            ]

            for jc in range(n_jc):
                for nt in range(n_nt):
                    b = pw_pool.tile([128, jc_w], U16, tag="pwb", name="pwb")
                    nc.gpsimd.dma_start(
                        out=b[:],
                        in_=pw[nt * 128:(nt + 1) * 128, jc * jc_w:(jc + 1) * jc_w],
                    )
                    wnat = wnat_pool.tile([128, 2 * jc_w], U16,
                                          tag="wnat", name="wnat")
                    _decode_chunk(nc.vector, dqv_pool, b, wnat, jc_w)
                    nc.gpsimd.dma_start(
                        out=wdram[jc][nt * 128:(nt + 1) * 128, :], in_=wnat[:]
                    )

            # ---- GEMM: two semi-passes over k ----
            for half_idx in range(2):
                p_lo, p_hi = half_idx * half, (half_idx + 1) * half
                for g in range(n_g):
                    psums = [
                        psum_pool.tile([128, n_shard], F32, tag="ps", name="ps")
                        for _ in range(mt_per_g)
                    ]
                    for p in range(p_lo, p_hi):
                        if g == 0:
                            # W transpose for this k-phase (slab becomes
                            # available as soon as its dequant lands)
                            hwdge[0].dma_start_transpose(
                                wt_slabs[p][:], wdram[p][:]
                            )
                        xts = []
                        for h in range(mt_per_g // 4):
                            xt = xt_pool.tile([128, kt_per_jc, 512], F16,
                                              tag="xt", name="xt")
                            m0 = g * (mt_per_g * 128) + h * 512
                            k0 = p * (kt_per_jc * 128)
                            hwdge[0].dma_start_transpose(
                                xt[:], x[m0:m0 + 512, k0:k0 + kt_per_jc * 128]
                            )
                            xts.append(xt)
                        for ml in range(mt_per_g):
                            for kl in range(kt_per_jc):
                                lhsT = xts[ml // 4][:, kl,
                                                    (ml % 4) * 128:(ml % 4 + 1) * 128]
                                nc.tensor.matmul(
                                    psums[ml][:],
                                    lhsT,
                                    wt_slabs[p][:, kl, :].bitcast(F16),
                                    start=(p == p_lo and kl == 0),
                                    stop=(p == p_hi - 1 and kl == kt_per_jc - 1),
                                )
                    if half_idx == 0:
                        # stage partial sums, freeing psum banks
                        for ml in range(mt_per_g):
                            nc.vector.tensor_copy(
                                out=stg[g][ml][:], in_=psums[ml][:]
                            )
                    else:
                        for ml in range(mt_per_g):
                            ot = out_pool.tile([128, n_shard], F16,
                                               tag="ot", name="ot")
                            nc.vector.tensor_tensor(
                                ot[:], psums[ml][:], stg[g][ml][:], op=alu.add
                            )
                            mt = g * mt_per_g + ml
                            nc.gpsimd.dma_start(
                                out=y[mt * 128:(mt + 1) * 128, :], in_=ot[:]
                            )
    nc.compile()
    return nc


_NC_CACHE = {}


def _run(x, packed_weight, **spmd_kwargs):
    key = "full"
    if key not in _NC_CACHE:
        _NC_CACHE[key] = build_nc()
    nc = _NC_CACHE[key]

    x = np.ascontiguousarray(np.asarray(x, dtype=np.float16))
    pw_u16 = np.asarray(packed_weight, dtype=np.int32).astype(np.uint16)
    in_maps = [
        {
            "x": x,
            "pw": np.ascontiguousarray(
                pw_u16[c * N_SHARD:(c + 1) * N_SHARD, :]
            ),
        }
        for c in range(N_CORES)
    ]
    res = run_bass_kernel_spmd(
        nc, in_maps, core_ids=list(range(N_CORES)), **spmd_kwargs
    )
    y = np.concatenate([res.results[c]["y"] for c in range(N_CORES)], axis=1)
    return y, res


def kernel(x, packed_weight, lut):
    assert np.allclose(np.asarray(lut, np.float32),
                       np.array(FP4_E2M1_VALUES, np.float32)), \
        "kernel's hardcoded decode only supports the standard table"
    y, _ = _run(x, packed_weight)
    return y
